# Initial kernel scaffold
#
"""Trainium2 Bass kernel for nn_Encoder_90632399880827 (gnn_message_passing).

Math (per frame of J=24 joints, SMPL tree):
  x  = relu(src @ W_pre + b_pre)                    (per node, 3 -> 24)
  h  = x @ W_gat  (24 -> 24, viewed as 3 heads x 8)
  a_src/a_dst = per-head dot(h, att)                (per node, 3 each)
  Each dst node j has exactly 2 incoming edges: self-loop (j) and parent
  p(j) (j=0: self-loop only).  Segment-softmax over 2 elements collapses to
      alpha = sigmoid(lrelu(u) - lrelu(v)),  u = a_src[j]+a_dst[j],
                                             v = a_src[p]+a_dst[j]
      out_j = relu( hpar' + alpha * (h'_j - hpar') ),   h' = h + b_gat,
  with p(0) := 0 so out_0 = relu(h'_0) exactly.

Layout (per core, data parallel over batch N: 64/8 = 8 batches/core):
  frames on SBUF partitions (128/tile), features on the free dim.
  stage1:  x (128, 768)  col 32j+m: m<24 x-feat, m=24 constant 1 (bias trick)
  stage2:  out2 (128, 768) col 32j+m: m<24 h', m=24+h a_src, m=27+h a_dst
  PE transposes provide the stationary (K-major) operands.
"""

import sys

if '/opt/trn_rl_repo' not in sys.path:
    sys.path.insert(0, '/opt/trn_rl_repo')

import numpy as np

J = 24
CIN = 3
HID = 24
HEADS = 3
O = 8
NEG = 0.2
N_BATCH, L, = 64, 2048
NCORES = 8
FRAMES_PER_CORE = (N_BATCH // NCORES) * L          # 16384
P = 128                                            # frames per tile
SMPL_PARENTS = np.array([-1, 0, 0, 0, 1, 2, 3, 4, 5, 6, 7, 8, 9, 9, 9, 12,
                         13, 14, 16, 17, 18, 19, 20, 21])
# parent of joint 0 -> itself (makes out_0 = h'_0 for any alpha)
PAR = SMPL_PARENTS.copy()
PAR[0] = 0

# gather runs over dst joints: (dst_start, n, src_start, src_step)
GATHER_RUNS = [
    (0, 4, 0, 0),      # j 0..3   <- 0
    (4, 9, 1, 1),      # j 4..12  <- 1..9
    (13, 2, 9, 0),     # j 13,14  <- 9
    (15, 3, 12, 1),    # j 15..17 <- 12..14
    (18, 6, 16, 1),    # j 18..23 <- 16..21
]


def _check_runs():
    par = np.empty(J, dtype=int)
    for d0, n, s0, st in GATHER_RUNS:
        for i in range(n):
            par[d0 + i] = s0 + st * i
    assert (par == PAR).all(), (par, PAR)


_check_runs()

# engine-assignment switches (bisectable)
CFG = {
    'gp_gather': True,    # gathers via scalar-issued sbuf->sbuf DMA
    'dve_relu_drain': True,  # xT relu drain alternates DVE/ACT
    'inplace_stt': True,  # lrelu in place on uv
    'dma_relu_out': False,  # final relu via SWDGE dma accum max vs zeros (BROKEN)
}


def _build_weights(W_pre, b_pre, W_gat, att_src, att_dst, b_gat):
    """Host-side block weight construction (float32)."""
    W_pre = np.asarray(W_pre, np.float32)
    b_pre = np.asarray(b_pre, np.float32)
    W_gat = np.asarray(W_gat, np.float32)
    att_src = np.asarray(att_src, np.float32)
    att_dst = np.asarray(att_dst, np.float32)
    b_gat = np.asarray(b_gat, np.float32)

    # a_src[n,h] = sum_o (x@W_gat)[n, h*O+o] * att_src[h,o]  = x @ w_as[:, h]
    w_as = np.zeros((HID, HEADS), np.float32)
    w_ad = np.zeros((HID, HEADS), np.float32)
    for h in range(HEADS):
        w_as[:, h] = W_gat[:, h * O:(h + 1) * O] @ att_src[h]
        w_ad[:, h] = W_gat[:, h * O:(h + 1) * O] @ att_dst[h]

    # stage 1: (73, 768): srcT rows (3j+c; row 72 = ones) -> x cols (32j+m)
    w1 = np.zeros((73, 768), np.float32)
    for j in range(J):
        w1[3 * j:3 * j + 3, 32 * j:32 * j + HID] = W_pre
        w1[72, 32 * j:32 * j + HID] = b_pre
        w1[72, 32 * j + 24] = 1.0          # constant-one column (post relu)

    # stage 2: (128, 768): 6 chunks of 4 joints.
    # xT chunk-c row 32g+k (k<24 x-feat of joint 4c+g, k=24 ones)
    #   -> out2 col 128c + 32g + m
    w2 = np.zeros((128, 768), np.float32)
    for c in range(6):
        for g in range(4):
            r = 32 * g
            m = 128 * c + 32 * g
            w2[r:r + HID, m:m + HID] = W_gat
            w2[24 + r, m:m + HID] = b_gat
            w2[r:r + HID, m + 24:m + 27] = w_as
            w2[r:r + HID, m + 27:m + 30] = w_ad
    return w1, w2


def _emit(nc, tc, ctx, srcn, w1d, w2d, outd, ntiles):
    """Emit the Tile program. srcn (FR,72) bf16, w1d (73,768) bf16,
    w2d (128,768) bf16, outd (FR,576) f32 DRAM APs."""
    import concourse.bass as bass
    from concourse import mybir
    from concourse.masks import make_identity
    F = mybir.ActivationFunctionType
    bf16 = mybir.dt.bfloat16
    f32 = mybir.dt.float32

    consts = ctx.enter_context(tc.tile_pool(name="consts", bufs=1))
    work = ctx.enter_context(tc.tile_pool(name="work", bufs=4))
    small = ctx.enter_context(tc.tile_pool(name="small", bufs=4))
    outp = ctx.enter_context(tc.tile_pool(name="outp", bufs=4))
    ps_x = ctx.enter_context(tc.tile_pool(name="ps_x", bufs=2, space="PSUM"))
    ps_o = ctx.enter_context(tc.tile_pool(name="ps_o", bufs=1, space="PSUM"))
    ps_tr = ctx.enter_context(tc.tile_pool(name="ps_tr", bufs=2, space="PSUM"))

    ident = consts.tile([P, P], bf16)
    make_identity(nc, ident)
    w1_sb = consts.tile([73, 768], bf16)
    nc.sync.dma_start(out=w1_sb, in_=w1d)
    w2_sb = consts.tile([128, 768], bf16)
    nc.sync.dma_start(out=w2_sb, in_=w2d)

    AL = mybir.AluOpType
    for it in range(ntiles):
        fr = it * P
        # ---- load + transpose src ----
        src_nat = work.tile([P, 73], bf16, tag="src_nat")
        nc.sync.dma_start(out=src_nat, in_=srcn[fr:fr + P, :])
        srcT_ps = ps_tr.tile([73, P], bf16, tag="tp_src")
        nc.tensor.transpose(srcT_ps, src_nat, ident)
        srcT = work.tile([73, P], bf16, tag="srcT")
        nc.vector.tensor_copy(out=srcT, in_=srcT_ps)

        # ---- stage 1 reversed: produce xT chunks directly ----
        # xT_ps col 128c+f = (within-chunk x-row, frame f) of chunk c
        xT_ps = ps_x.tile([P, 768], f32, tag="xT_ps")
        for c in range(6):
            nc.tensor.matmul(xT_ps[:, 128 * c:128 * (c + 1)],
                             lhsT=w1_sb[:, 128 * c:128 * (c + 1)], rhs=srcT,
                             start=True, stop=True)
        xT = work.tile([P, 768], bf16, tag="xT")
        nc.scalar.activation(xT, xT_ps, F.Relu)

        # ---- stage 2 ----
        out2_ps = ps_o.tile([P, 768], f32, tag="out2_ps")
        for c in range(6):
            nc.tensor.matmul(out2_ps[:, 128 * c:128 * (c + 1)],
                             lhsT=xT[:, 128 * c:128 * (c + 1)],
                             rhs=w2_sb[:, 128 * c:128 * (c + 1)],
                             start=True, stop=True)
        out2 = work.tile([P, 768], bf16, tag="out2")
        nc.scalar.activation(out2, out2_ps, F.Copy)

        o2 = out2.rearrange("p (j m) -> p j m", m=32)
        h_cols = o2[:, :, 0:24]                     # (128,24,24) h'
        s_cols = o2[:, :, 24:27]                    # (128,24,3) a_src
        t_cols = o2[:, :, 27:30]                    # (128,24,3) a_dst

        # ---- gather parent h' and a_src (27 cols per joint) ----
        hs_par = small.tile([P, J, 27], bf16, tag="hs_par")
        for i, (d0, n, s0, st) in enumerate(GATHER_RUNS):
            src_ap = bass.AP(
                tensor=out2.tensor, offset=out2.offset + 32 * s0,
                ap=[out2.ap[0], [32 * st, n], [1, 27]])
            if CFG['gp_gather']:
                nc.scalar.dma_start(out=hs_par[:, d0:d0 + n, :], in_=src_ap)
            else:
                nc.vector.tensor_copy(out=hs_par[:, d0:d0 + n, :], in_=src_ap)
        hpar = hs_par[:, :, 0:24]
        spar = hs_par[:, :, 24:27]

        # ---- attention coefficients: uv = [u | v], lrelu, d, sigmoid ----
        uv = small.tile([P, 2, J, HEADS], bf16, tag="uv")
        nc.gpsimd.tensor_add(uv[:, 0], s_cols, t_cols)
        nc.gpsimd.tensor_add(uv[:, 1], spar, t_cols)
        uv_flat = uv.rearrange("p a j h -> p (a j h)")
        if CFG['inplace_stt']:
            nc.vector.scalar_tensor_tensor(uv_flat, in0=uv_flat, scalar=NEG,
                                           in1=uv_flat, op0=AL.mult, op1=AL.max)
        else:
            euv = small.tile([P, 2, J, HEADS], bf16, tag="euv")
            ef = euv.rearrange("p a j h -> p (a j h)")
            nc.vector.scalar_tensor_tensor(ef, in0=uv_flat, scalar=NEG,
                                           in1=uv_flat, op0=AL.mult, op1=AL.max)
            uv = euv
        d = small.tile([P, J, HEADS], bf16, tag="d")
        nc.vector.tensor_sub(d, uv[:, 0], uv[:, 1])
        alpha_bc = small.tile([P, J, HEADS, O], bf16, tag="alpha_bc")
        nc.scalar.activation(alpha_bc,
                             d.unsqueeze(3).broadcast_to((P, J, HEADS, O)),
                             F.Sigmoid)

        # ---- aggregate: out = relu(hpar + alpha*(h' - hpar)) ----
        g = work.tile([P, J, HID], bf16, tag="g")
        nc.vector.tensor_sub(g, h_cols, hpar)
        prod = work.tile([P, J, HID], bf16, tag="prod")
        nc.vector.tensor_mul(
            prod.rearrange("p j m -> p (j m)"),
            g.rearrange("p j m -> p (j m)"),
            alpha_bc.rearrange("p j h o -> p (j h o)"))
        agg = work.tile([P, J, HID], bf16, tag="agg")
        nc.vector.tensor_add(agg, prod, hpar)
        if CFG['dma_relu_out']:
            nc.gpsimd.dma_start(out=outd[fr:fr + P, :],
                                in_=agg.rearrange("p j m -> p (j m)"),
                                accum_op=AL.max)
        else:
            outf = outp.tile([P, J * HID], f32, tag="outf")
            nc.gpsimd.tensor_scalar_max(
                out=outf, in0=agg.rearrange("p j m -> p (j m)"),
                scalar1=0.0)
            nc.sync.dma_start(out=outd[fr:fr + P, :], in_=outf)


def build_program(ntiles=FRAMES_PER_CORE // P):
    import concourse.bacc as bacc
    import concourse.tile as tile
    from concourse import mybir
    from contextlib import ExitStack

    FR = ntiles * P
    nc = bacc.Bacc("TRN2", num_devices=NCORES)
    srcn = nc.dram_tensor("srcn", (FR, 73), mybir.dt.bfloat16,
                          kind="ExternalInput")
    w1d = nc.dram_tensor("w1", (73, 768), mybir.dt.bfloat16,
                         kind="ExternalInput")
    w2d = nc.dram_tensor("w2", (128, 768), mybir.dt.bfloat16,
                         kind="ExternalInput")
    outd = nc.dram_tensor("out", (FR, 576), mybir.dt.float32,
                          kind="ExternalOutput")
    with tile.TileContext(nc) as tc:
        with ExitStack() as ctx:
            _emit(nc, tc, ctx, srcn[:, :], w1d[:, :], w2d[:, :], outd[:, :],
                  ntiles)
    nc.finalize()
    return nc


def _to_bf16(a):
    import ml_dtypes
    return np.asarray(a, np.float32).astype(ml_dtypes.bfloat16)


def _run(src, W_pre, b_pre, W_gat, att_src, att_dst, b_gat, **spmd_kwargs):
    import sys
    if '/opt/trn_rl_repo' not in sys.path:
        sys.path.insert(0, '/opt/trn_rl_repo')
    from concourse.bass_utils import run_bass_kernel_spmd

    src = np.asarray(src, np.float32)
    n, l, dd = src.shape
    w1, w2 = _build_weights(W_pre, b_pre, W_gat, att_src, att_dst, b_gat)
    w1b, w2b = _to_bf16(w1), _to_bf16(w2)

    per = n // NCORES
    shards = src.reshape(NCORES, per * l, dd)
    ones = np.ones((per * l, 1), np.float32)
    in_maps = [{
        "srcn": _to_bf16(np.concatenate([shards[i], ones], axis=1)),
        "w1": w1b,
        "w2": w2b,
    } for i in range(NCORES)]

    nc = build_program(per * l // P)
    res = run_bass_kernel_spmd(nc, in_maps, list(range(NCORES)), **spmd_kwargs)
    out = np.stack([res.results[i]["out"] for i in range(NCORES)])
    return out.reshape(n, l, J * HID).astype(np.float32), res


def kernel(src, W_pre, b_pre, W_gat, att_src, att_dst, b_gat):
    out, _ = _run(src, W_pre, b_pre, W_gat, att_src, att_dst, b_gat)
    return out


def kernel_traced(src, W_pre, b_pre, W_gat, att_src, att_dst, b_gat):
    return _run(src, W_pre, b_pre, W_gat, att_src, att_dst, b_gat, trace=True)


if __name__ == "__main__":
    # quick numeric self-check of the host-side math vs a numpy model
    rng = np.random.default_rng(0)
    pass



# revision 2
# speedup vs baseline: 1.1768x; 1.1768x over previous
"""Trainium2 Bass kernel for nn_Encoder_90632399880827 (gnn_message_passing).

Math (per frame of J=24 joints, SMPL tree, parent p(j), p(0):=0):
  x   = relu(src @ W_pre + b_pre)            (per node, 3 -> 24)
  h   = x @ W_gat                            (24 -> 24, 3 heads x 8)
  u_j = h_j.(att_src+att_dst)  (per head)    v_j = h_p.att_src + h_j.att_dst
  d   = lrelu(u) - lrelu(v);  alpha = sigmoid(d)
  out_j = relu( hpar_j + alpha_j * g_j + b_gat ),  hpar = h_p, g = h_j - h_p

Layout: frames on partitions (128/tile), features on the free dim.
Joints are packed 5-per-K-chunk (24 x-rows each + shared ones row at 120);
chunk groups minimize cross-chunk parent edges (6 cross joints). Stage-2
matmul weights compute hpar/g/u/v DIRECTLY (the tree gather lives in the
weights; cross joints accumulate over two K chunks in PSUM).

out2 psum layout: 3 banks x 8 joints x 54 cols [u 3 | v 3 | hpar 24 | g 24]
joint order ORDER (bank-major) chosen so every matmul col-run stays inside
one 2KB psum bank.  Host un-permutes joints at the end.
"""

import sys

if '/opt/trn_rl_repo' not in sys.path:
    sys.path.insert(0, '/opt/trn_rl_repo')

import numpy as np

J = 24
CIN = 3
HID = 24
HEADS = 3
O = 8
NEG = 0.2
N_BATCH, L = 64, 2048
NCORES = 8
FRAMES_PER_CORE = (N_BATCH // NCORES) * L          # 16384
P = 128                                            # frames per tile
SMPL_PARENTS = np.array([-1, 0, 0, 0, 1, 2, 3, 4, 5, 6, 7, 8, 9, 9, 9, 12,
                         13, 14, 16, 17, 18, 19, 20, 21])
PAR = SMPL_PARENTS.copy()
PAR[0] = 0

# K-chunk joint groups (5 chunks; <=5 joints of 24 x-rows + ones row @120)
CHUNKS = [
    [0, 1, 4, 7, 10],
    [2, 3, 5, 8, 11],
    [6, 9, 12, 13, 15],
    [14, 16, 18, 20, 22],
    [17, 19, 21, 23],
]
NCHUNK = len(CHUNKS)
ONES_ROW = 120                       # per-chunk constant-one x-row

CHUNK_OF = {}
POS_OF = {}
for _c, _js in enumerate(CHUNKS):
    for _g, _j in enumerate(_js):
        CHUNK_OF[_j] = _c
        POS_OF[_j] = _g

# out2 joint order: 3 banks x 8 joints; every matmul col-run within a bank.
# same-chunk joints grouped per chunk; cross joints grouped per (a,b) pair.
SAME = {c: [j for j in CHUNKS[c] if CHUNK_OF[PAR[j]] == c] for c in range(NCHUNK)}
CROSS_PAIRS = [(1, 0, [2, 3]), (2, 1, [6]), (3, 2, [14, 16]), (4, 3, [17])]
ORDER = (SAME[0] + SAME[1]                # bank 0: 5 + 3 = 8
         + SAME[2] + SAME[3] + [6]       # bank 1: 4 + 3 + 1 = 8
         + SAME[4] + [2, 3] + [14, 16] + [17])   # bank 2: 3+2+2+1 = 8
assert sorted(ORDER) == list(range(J)), ORDER
POSO = {j: i for i, j in enumerate(ORDER)}           # out2 position of joint

JSTRIDE = 54                 # cols per joint in out2 [u3|v3|hpar24|g24]
BANKF32 = 512                # f32 cols per psum bank
C2TOT = 3 * BANKF32          # out2 free size (f32)


def _jcol(j):
    k = POSO[j]
    return BANKF32 * (k // 8) + JSTRIDE * (k % 8)


# stage-2 matmul plan: (out_col_start, njoints, lhs_chunk, role, joints)
# role 'same': single-shot;  cross pairs emit ('a', start) + ('b', stop).
def _mm_plan():
    plan = []   # (out_start, ncols, lhs_chunk, kind, joints, start, stop)
    def run(joints, lhs_chunk, kind, start, stop):
        k0 = POSO[joints[0]]
        assert [POSO[j] for j in joints] == list(range(k0, k0 + len(joints)))
        assert (k0 // 8) == ((k0 + len(joints) - 1) // 8), "bank straddle"
        plan.append((_jcol(joints[0]), len(joints) * JSTRIDE, lhs_chunk,
                     kind, tuple(joints), start, stop))
    for c in range(NCHUNK):
        run(SAME[c], c, 'same', True, True)
    for a, b, js in CROSS_PAIRS:
        run(js, a, 'xa', True, False)
        run(js, b, 'xb', False, True)
    return plan


MM_PLAN = _mm_plan()
W2COLS = sum(p[1] for p in MM_PLAN)                  # 1620


# engine-assignment switches (bisectable)
CFG = {
    # uv lrelu: 'prelu' = ACT parametric_relu fused with the uv drain (same
    # act table as relu/sigmoid/copy; unsupported by CoreSim — HW only);
    # 'dve2' = ACT copy drain + 2 DVE ops (sim-safe).  Never DVE stt: the
    # MULTIPLY,MAX microcode measured 6.5us/tile on HW.
    'lrelu_mode': 'prelu',
    'bc_sigmoid': False,      # ACT broadcast sigmoid (else narrow + bc mul)
    'add_split_pool': 0,     # joints of the final add on Pool (0..24)
    'relu_engine': 'dve',     # final relu: 'dve' | 'pool' | 'act'
    'out_dma_engine': 'gpsimd',  # final store issue engine
    'supertile': 1,           # tiles per elementwise batch (psum permitting)
}


def _build_weights(W_pre, b_pre, W_gat, att_src, att_dst, b_gat):
    """w1 (73, 640): srcT rows (3j+c; 72=ones) -> xT chunk cols.
    w2 (128, W2COLS): per-instruction blocks, rows = chunk x-rows."""
    W_pre = np.asarray(W_pre, np.float32)
    b_pre = np.asarray(b_pre, np.float32)
    W_gat = np.asarray(W_gat, np.float32)
    att_src = np.asarray(att_src, np.float32)
    att_dst = np.asarray(att_dst, np.float32)
    b_gat = np.asarray(b_gat, np.float32)

    w_as = np.zeros((HID, HEADS), np.float32)
    w_ad = np.zeros((HID, HEADS), np.float32)
    for h in range(HEADS):
        w_as[:, h] = W_gat[:, h * O:(h + 1) * O] @ att_src[h]
        w_ad[:, h] = W_gat[:, h * O:(h + 1) * O] @ att_dst[h]

    w1 = np.zeros((73, 128 * NCHUNK), np.float32)
    for c, js in enumerate(CHUNKS):
        for g, j in enumerate(js):
            col = 128 * c + 24 * g
            w1[3 * j:3 * j + 3, col:col + HID] = W_pre
            w1[72, col:col + HID] = b_pre
        w1[72, 128 * c + ONES_ROW] = 1.0

    # w2: one 128-row block per matmul instruction, columns appended in
    # MM_PLAN order.  Rows: joint at pos g -> 24g..24g+23; ones @120.
    w2 = np.zeros((128, W2COLS), np.float32)
    off = 0
    for (out0, ncols, lc, kind, joints, start, stop) in MM_PLAN:
        blk = w2[:, off:off + ncols]
        for i, j in enumerate(joints):
            p = PAR[j]
            ro = 24 * POS_OF[j]              # own rows (in chunk CHUNK_OF[j])
            rp = 24 * POS_OF[p]              # parent rows (in chunk of p)
            cu, cv, ch, cg = (JSTRIDE * i, JSTRIDE * i + 3,
                              JSTRIDE * i + 6, JSTRIDE * i + 30)
            if kind in ('same', 'xa'):
                assert lc == CHUNK_OF[j]
                blk[ro:ro + HID, cu:cu + 3] = w_as + w_ad        # u
                blk[ro:ro + HID, cv:cv + 3] = w_ad               # v own part
                blk[ro:ro + HID, cg:cg + HID] = W_gat            # g own part
            if kind in ('same', 'xb'):
                assert lc == CHUNK_OF[p]
                blk[rp:rp + HID, cv:cv + 3] += w_as              # v parent
                blk[rp:rp + HID, ch:ch + HID] += W_gat           # hpar
                blk[ONES_ROW, ch:ch + HID] += b_gat
                blk[rp:rp + HID, cg:cg + HID] -= W_gat           # g parent
        off += ncols
    return w1, w2


def _np_pipeline(srcT, w1, w2):
    """Numpy model of the device program (f32). srcT (73, F)."""
    F = srcT.shape[1]
    xT = np.zeros((128, NCHUNK * F), np.float32)
    for c in range(NCHUNK):
        xT[:, c * F:(c + 1) * F] = np.maximum(w1[:, 128 * c:128 * (c + 1)].T
                                              @ srcT, 0.0)
    out2 = np.zeros((F, C2TOT), np.float32)
    off = 0
    for (out0, ncols, lc, kind, joints, start, stop) in MM_PLAN:
        acc = xT[:, lc * F:(lc + 1) * F].T @ w2[:, off:off + ncols]
        if start:
            out2[:, out0:out0 + ncols] = acc
        else:
            out2[:, out0:out0 + ncols] += acc
        off += ncols
    # elementwise
    out = np.zeros((F, J, HID), np.float32)
    for j in range(J):
        q = _jcol(j)
        u = out2[:, q:q + 3]
        v = out2[:, q + 3:q + 6]
        hpar = out2[:, q + 6:q + 30]
        g = out2[:, q + 30:q + 54]
        d = (np.maximum(NEG * u, u) - np.maximum(NEG * v, v))
        alpha = 1.0 / (1.0 + np.exp(-d))                      # (F,3)
        ab = np.repeat(alpha, O, axis=1)                      # (F,24)
        out[:, POSO[j]] = np.maximum(hpar + ab * g, 0.0)      # device order
    return out.reshape(F, J * HID)


def _emit(nc, tc, ctx, srcn, w1d, w2d, outd, ntiles):
    """srcn (73, FR) bf16, w1d (73,640) bf16, w2d (128,W2COLS) bf16,
    outd (FR, 576) bf16 DRAM APs.  out joint order = ORDER (host fixes)."""
    import concourse.bass as bass
    from concourse import mybir
    F = mybir.ActivationFunctionType
    AL = mybir.AluOpType
    bf16 = mybir.dt.bfloat16
    f32 = mybir.dt.float32

    consts = ctx.enter_context(tc.tile_pool(name="consts", bufs=1))
    work = ctx.enter_context(tc.tile_pool(name="work", bufs=4))
    small = ctx.enter_context(tc.tile_pool(name="small", bufs=4))
    outp = ctx.enter_context(tc.tile_pool(name="outp", bufs=4))
    ps_x = ctx.enter_context(tc.tile_pool(name="ps_x", bufs=1, space="PSUM"))
    ps_o = ctx.enter_context(tc.tile_pool(name="ps_o", bufs=2, space="PSUM"))

    w1_sb = consts.tile([73, 128 * NCHUNK], bf16)
    nc.sync.dma_start(out=w1_sb, in_=w1d)
    w2_sb = consts.tile([128, W2COLS], bf16)
    nc.sync.dma_start(out=w2_sb, in_=w2d)

    for it in range(ntiles):
        fr = it * P
        # ---- load srcT slice (73, 128) ----
        srcT = work.tile([73, P], bf16, tag="srcT")
        nc.sync.dma_start(out=srcT, in_=srcn[:, fr:fr + P])

        # ---- stage 1: xT chunks (chunk rows on partitions, frames free) ----
        xT_ps = ps_x.tile([P, 128 * NCHUNK], f32, tag="xT_ps")
        for c in range(NCHUNK):
            nc.tensor.matmul(xT_ps[:, P * c:P * (c + 1)],
                             lhsT=w1_sb[:, 128 * c:128 * (c + 1)], rhs=srcT,
                             start=True, stop=True)
        xT = work.tile([P, 128 * NCHUNK], bf16, tag="xT")
        nc.scalar.activation(xT, xT_ps, F.Relu)

        # ---- stage 2: out2 = [u|v|hpar|g] per joint, tree in the weights --
        out2 = ps_o.tile([P, C2TOT], f32, tag="out2")
        off = 0
        for (out0, ncols, lc, kind, joints, start, stop) in MM_PLAN:
            nc.tensor.matmul(out2[:, out0:out0 + ncols],
                             lhsT=xT[:, P * lc:P * (lc + 1)],
                             rhs=w2_sb[:, off:off + ncols],
                             start=start, stop=stop)
            off += ncols

        o2 = bass.AP(tensor=out2.tensor, offset=out2.offset,
                     ap=[out2.ap[0], [BANKF32, 3], [JSTRIDE, 8], [1, JSTRIDE]])

        # ---- attention: d = lrelu(u) - lrelu(v); alpha = sigmoid(d) ------
        uv_ap = bass.AP(tensor=out2.tensor, offset=out2.offset,
                        ap=[out2.ap[0], [BANKF32, 3], [JSTRIDE, 8], [1, 6]])
        # drain uv to SBUF (hw: at most ONE psum input per instruction)
        uvl = small.tile([P, J, 6], bf16, tag="uvl")
        if CFG['lrelu_mode'] == 'prelu':
            nc.scalar.activation(uvl, uv_ap, F.Prelu, alpha=NEG)
        else:
            uvs = small.tile([P, J, 6], bf16, tag="uvs")
            nc.scalar.activation(uvs, uv_ap, F.Copy)
            uvsf = uvs.rearrange("p j s -> p (j s)")
            uvf = uvl.rearrange("p j s -> p (j s)")
            nc.vector.tensor_scalar_mul(out=uvf, in0=uvsf, scalar1=NEG)
            nc.vector.tensor_max(uvf, uvf, uvsf)
        d = small.tile([P, J, HEADS], bf16, tag="d")
        nc.vector.tensor_sub(d, uvl[:, :, 0:3], uvl[:, :, 3:6])

        # ---- alpha & combine ------------------------------------------------
        hpar_ap = bass.AP(tensor=out2.tensor, offset=out2.offset + 6,
                          ap=[out2.ap[0], [BANKF32, 3], [JSTRIDE, 8], [1, HID]])
        g_ap = bass.AP(tensor=out2.tensor, offset=out2.offset + 30,
                       ap=[out2.ap[0], [BANKF32, 3], [JSTRIDE, 8], [1, HID]])
        prod = work.tile([P, J, HID], bf16, tag="prod")
        if CFG['bc_sigmoid']:
            alpha_bc = work.tile([P, J, HEADS, O], bf16, tag="alpha_bc")
            nc.scalar.activation(alpha_bc,
                                 d.unsqueeze(3).broadcast_to((P, J, HEADS, O)),
                                 F.Sigmoid)
            nc.vector.tensor_mul(prod.rearrange("p j m -> p (j m)"), g_ap,
                                 alpha_bc.rearrange("p j h o -> p (j h o)"))
        else:
            alpha = small.tile([P, J, HEADS], bf16, tag="alpha")
            nc.scalar.activation(alpha, d, F.Sigmoid)
            nc.vector.tensor_mul(
                prod.rearrange("p j m -> p (j m)"), g_ap,
                alpha.unsqueeze(3).broadcast_to((P, J, HEADS, O)))

        outf = outp.tile([P, J * HID], bf16, tag="outf")
        of3 = outf.rearrange("p (j m) -> p j m", m=HID)
        nsp = CFG['add_split_pool']          # pool joints (multiple of 8)
        assert nsp % 8 == 0
        ndve = J - nsp                       # leading joints (banks) on DVE
        if ndve > 0:
            hp = bass.AP(tensor=out2.tensor, offset=out2.offset + 6,
                         ap=[out2.ap[0], [BANKF32, ndve // 8],
                             [JSTRIDE, 8], [1, HID]])
            nc.vector.tensor_add(of3[:, 0:ndve], prod[:, 0:ndve], hp)
        if nsp > 0:
            hp = bass.AP(tensor=out2.tensor,
                         offset=out2.offset + BANKF32 * (ndve // 8) + 6,
                         ap=[out2.ap[0], [BANKF32, nsp // 8],
                             [JSTRIDE, 8], [1, HID]])
            nc.gpsimd.tensor_add(of3[:, ndve:J], prod[:, ndve:J], hp)

        if CFG['relu_engine'] == 'dve':
            nc.vector.tensor_scalar_max(out=outf, in0=outf, scalar1=0.0)
        elif CFG['relu_engine'] == 'pool':
            nc.gpsimd.tensor_scalar_max(out=outf, in0=outf, scalar1=0.0)
        else:
            nc.scalar.activation(outf, outf, F.Relu)

        eng = getattr(nc, CFG['out_dma_engine'])
        eng.dma_start(out=outd[fr:fr + P, :], in_=outf)


def build_program(ntiles=FRAMES_PER_CORE // P, ndev=NCORES):
    import concourse.bacc as bacc
    import concourse.tile as tile
    from concourse import mybir
    from contextlib import ExitStack

    FR = ntiles * P
    nc = bacc.Bacc("TRN2", num_devices=ndev)
    srcn = nc.dram_tensor("srcn", (73, FR), mybir.dt.bfloat16,
                          kind="ExternalInput")
    w1d = nc.dram_tensor("w1", (73, 128 * NCHUNK), mybir.dt.bfloat16,
                         kind="ExternalInput")
    w2d = nc.dram_tensor("w2", (128, W2COLS), mybir.dt.bfloat16,
                         kind="ExternalInput")
    outd = nc.dram_tensor("out", (FR, J * HID), mybir.dt.bfloat16,
                          kind="ExternalOutput")
    with tile.TileContext(nc) as tc:
        with ExitStack() as ctx:
            _emit(nc, tc, ctx, srcn[:, :], w1d[:, :], w2d[:, :], outd[:, :],
                  ntiles)
    nc.finalize()
    return nc


def _to_bf16(a):
    import ml_dtypes
    return np.asarray(a, np.float32).astype(ml_dtypes.bfloat16)


def _host_inputs(src, W_pre, b_pre, W_gat, att_src, att_dst, b_gat):
    src = np.asarray(src, np.float32)
    n, l, dd = src.shape
    w1, w2 = _build_weights(W_pre, b_pre, W_gat, att_src, att_dst, b_gat)
    w1b, w2b = _to_bf16(w1), _to_bf16(w2)
    per = n // NCORES
    shards = src.reshape(NCORES, per * l, dd)
    in_maps = []
    for i in range(NCORES):
        st = np.empty((73, per * l), np.float32)
        st[:72] = shards[i].T
        st[72] = 1.0
        in_maps.append({"srcn": _to_bf16(st), "w1": w1b, "w2": w2b})
    return in_maps


def _unpermute(out_flat):
    """(..., 576) device order (ORDER-major) -> natural joint order."""
    shp = out_flat.shape[:-1]
    raw = out_flat.reshape(*shp, J, HID)
    nat = np.empty_like(raw)
    nat[..., np.array(ORDER), :] = raw
    return nat.reshape(*shp, J * HID)


def _run(src, W_pre, b_pre, W_gat, att_src, att_dst, b_gat, **spmd_kwargs):
    from concourse.bass_utils import run_bass_kernel_spmd
    n, l, dd = np.asarray(src).shape
    in_maps = _host_inputs(src, W_pre, b_pre, W_gat, att_src, att_dst, b_gat)
    per = n // NCORES
    nc = build_program(per * l // P)
    res = run_bass_kernel_spmd(nc, in_maps, list(range(NCORES)), **spmd_kwargs)
    out = np.stack([np.asarray(res.results[i]["out"], np.float32)
                    for i in range(NCORES)])
    out = _unpermute(out)
    return out.reshape(n, l, J * HID), res


def kernel(src, W_pre, b_pre, W_gat, att_src, att_dst, b_gat):
    out, _ = _run(src, W_pre, b_pre, W_gat, att_src, att_dst, b_gat)
    return out


def kernel_traced(src, W_pre, b_pre, W_gat, att_src, att_dst, b_gat):
    return _run(src, W_pre, b_pre, W_gat, att_src, att_dst, b_gat, trace=True)
